# revision 2
# baseline (speedup 1.0000x reference)
"""Causal multi-head attention layer (B=2, T=2048, C=2048, H=16) on 8 TRN2
NeuronCores — v4: fp16 + quad-strip attention + DVE softmax-denominator
accumulation.  (v1 fp32r: 487675ns; v2 bf16: 454541ns; v3: 418466ns.)

Sharding: data-parallel over batch (2 groups of 4 cores), tensor-parallel over
heads within a group (4 heads/core, Megatron column-split of w_attn and
row-split of w_proj).  Each core computes a partial projection output in
transposed layout yT = (O_heads @ w_proj[:, cols].T).T; the host transposes,
sums the 4 partials per batch element and adds b_proj.

v4 changes vs v3:
  - all operands fp16 (10-bit mantissa beats bf16's 8 for accuracy; same
    speed on PE, and unlocks DVE 2x 2-byte modes).
  - attention processes all 4 query strips of a head at once: kt/vt weight
    loads amortize over up to 4 matmuls each (vs 2 in the v3 pair scheme).
  - the per-chunk ones-matmul row-sum (69632 PE cycles + 96 LDWEIGHTS) is
    replaced by DVE accumulation of exp chunks into R[s] (fp16, 2x mode) +
    ONE ones-matmul per (head, strip) on the accumulated R.
  - causal diagonal mask applied by multiplying exp output with a 0/1
    fp16 mask on DVE (128 cols) instead of adding -1e30 to f32 psum.
  - q/k/v psum->sbuf copies moved to the (otherwise idle) Act engine.
  - q/k passes run in u-pairs (8 psum accumulators) so the first x sweep
    takes ~28us against the ~30us x DMA, shrinking the startup stall;
    w_k/w_v chunk DMAs deferred until after x + w_q.
  - PSUM split into two 4-bank rings: "otp" (PV accumulators / proj) and
    "ps" (S tiles, row-sum output, qk/v accumulators).
"""

import numpy as np

import concourse.bacc as bacc
import concourse.tile as tile
from concourse import mybir
from concourse.bass_utils import run_bass_kernel_spmd

F32 = mybir.dt.float32
FP16 = mybir.dt.float16

B, T, C, H = 2, 2048, 2048, 16
HD = C // H            # 128
HLOC = 4               # heads per core
NCORES = 8
NSTRIP = T // 512      # 4 t-strips
NCH = C // 128         # 16 contraction chunks
SCALE = 1.0 / float(np.sqrt(HD))

_cache = {}


def _build_nc(reps=1):
    nc = bacc.Bacc("TRN2", debug=False)

    xt = nc.dram_tensor("xt", [C, T], FP16, kind="ExternalInput")      # x[b].T
    wqkv = nc.dram_tensor("wqkv", [C, 3 * 512], FP16, kind="ExternalInput")
    wp = nc.dram_tensor("wp", [512, C], FP16, kind="ExternalInput")
    mask01_in = nc.dram_tensor("mask01_in", [128, 128], FP16,
                               kind="ExternalInput")
    ones_in = nc.dram_tensor("ones_in", [128, 128], FP16, kind="ExternalInput")
    yt = nc.dram_tensor("yt", [C, T], FP16, kind="ExternalOutput")

    with tile.TileContext(nc) as tc:
        with (
            tc.tile_pool(name="persist", bufs=1) as persist,
            tc.tile_pool(name="work", bufs=2) as work,
            tc.tile_pool(name="psum", bufs=4, space="PSUM") as psum,
        ):
            qt = persist.tile([128, HLOC * T], FP16, tag="qt")
            kt = persist.tile([128, HLOC * T], FP16, tag="kt")
            vt = persist.tile([128, HLOC * T], FP16, tag="vt")
            ot = persist.tile([128, HLOC * T], FP16, tag="ot")
            mask01 = persist.tile([128, 128], FP16, tag="mask01")
            ones = persist.tile([128, 128], FP16, tag="ones")
            nc.sync.dma_start(out=mask01, in_=mask01_in[:, :])
            nc.sync.dma_start(out=ones, in_=ones_in[:, :])

            if reps > 1:
                loop_ctx = tc.For_i(
                    0, reps, 1,
                    hint_engines=(mybir.EngineType.PE,
                                  mybir.EngineType.DVE,
                                  mybir.EngineType.Activation,
                                  mybir.EngineType.SP,
                                  mybir.EngineType.Pool))
                loop_ctx.__enter__()

            # ---- DMA: x + w_q interleaved first (q pass is DMA-gated),
            # then w_k, then w_v ----
            xcs = {}
            wts = {}
            for cc in range(NCH):
                wt = work.tile([128, 512], FP16, tag="wch", bufs=32,
                               name=f"w_0_{cc}")
                nc.sync.dma_start(
                    out=wt, in_=wqkv[128 * cc:128 * (cc + 1), 0:512])
                wts[(0, cc)] = wt
                for s in range(NSTRIP):
                    xc = work.tile([128, 512], FP16, tag="xc", bufs=64,
                                   name=f"xc_{cc}_{s}")
                    nc.sync.dma_start(
                        out=xc, in_=xt[128 * cc:128 * (cc + 1),
                                       512 * s:512 * s + 512])
                    xcs[(cc, s)] = xc
            for pas in (1, 2):
                for cc in range(NCH):
                    wt = work.tile([128, 512], FP16, tag="wch", bufs=32,
                                   name=f"w_{pas}_{cc}")
                    nc.sync.dma_start(
                        out=wt, in_=wqkv[128 * cc:128 * (cc + 1),
                                         512 * pas:512 * (pas + 1)])
                    wts[(pas, cc)] = wt

            # ---- q/k: weight-stationary, u-pairs, full-K psum accumulation
            for pas in range(2):            # 0=q, 1=k
                dst = qt if pas == 0 else kt
                for up in range(2):         # u-pair = 2 head-blocks at a time
                    accs = {}
                    for u2 in range(2):
                        for s in range(NSTRIP):
                            tg = "otp" if u2 == 0 else "ps"
                            accs[(u2, s)] = psum.tile(
                                [128, 512], F32, tag=tg, bufs=4,
                                name=f"acc_{pas}_{up}_{u2}_{s}")
                    for cc in range(NCH):
                        for u2 in range(2):
                            u = 2 * up + u2
                            w_u = wts[(pas, cc)][:, 128 * u:128 * (u + 1)]
                            for s in range(NSTRIP):
                                nc.tensor.matmul(
                                    accs[(u2, s)], lhsT=w_u, rhs=xcs[(cc, s)],
                                    start=(cc == 0), stop=(cc == NCH - 1))
                    for u2 in range(2):
                        u = 2 * up + u2
                        for s in range(NSTRIP):
                            nc.scalar.copy(
                                dst[:, T * u + 512 * s:T * u + 512 * (s + 1)],
                                accs[(u2, s)])

            # ---- v: [tokens, vchan] orientation (x slices stationary) ----
            for s in range(NSTRIP):
                for u4 in range(4):         # token 128-block within strip
                    j = 4 * s + u4
                    acc = psum.tile([128, 512], F32, tag="ps", bufs=4,
                                    name=f"accv_{j}")
                    for cc in range(NCH):
                        nc.tensor.matmul(
                            acc,
                            lhsT=xcs[(cc, s)][:, 128 * u4:128 * (u4 + 1)],
                            rhs=wts[(2, cc)],
                            start=(cc == 0), stop=(cc == NCH - 1))
                    nc.scalar.copy(vt[:, 512 * j:512 * (j + 1)], acc)

            # ---- proj weights: DMA early so they arrive during attention ----
            wpt = {}
            for hp in range(HLOC):
                for cs in range(4):
                    wt = work.tile([128, 512], FP16, tag="xc", bufs=64,
                                   name=f"wpt_{hp}_{cs}")
                    nc.sync.dma_start(
                        out=wt, in_=wp[128 * hp:128 * (hp + 1),
                                       512 * cs:512 * (cs + 1)])
                    wpt[(hp, cs)] = wt

            # ---- attention: all 4 strips of a head at once ----
            for h in range(HLOC):
                otp = [psum.tile([128, 512], F32, tag="otp", bufs=4,
                                 name=f"otp_{h}_{s}")
                       for s in range(NSTRIP)]
                rts = [work.tile([128, 512], FP16, tag="rt", bufs=6,
                                 name=f"r_{h}_{s}")
                       for s in range(NSTRIP)]

                def emit_s(j):
                    """S matmuls + exp (+ diag mask) for chunk j, all strips."""
                    smin = j // 4
                    pts = {}
                    kslice = kt[:, T * h + 128 * j:T * h + 128 * (j + 1)]
                    for s in range(smin, NSTRIP):
                        o = 128 * (j - 4 * s) if s == smin else 0
                        t0 = 512 * s
                        stp = psum.tile([128, 512], F32, tag="ps", bufs=4,
                                        name=f"stp_{h}_{j}_{s}")
                        nc.tensor.matmul(
                            stp[:, o:], lhsT=kslice,
                            rhs=qt[:, T * h + t0 + o:T * h + t0 + 512],
                            start=True, stop=True)
                        pt = work.tile([128, 512], FP16, tag="pt", bufs=8,
                                       name=f"pt_{h}_{j}_{s}")
                        nc.scalar.activation(
                            pt[:, o:], stp[:, o:],
                            mybir.ActivationFunctionType.Exp,
                            scale=SCALE)
                        if s == smin:
                            nc.vector.tensor_mul(
                                pt[:, o:o + 128], pt[:, o:o + 128], mask01)
                        pts[s] = (pt, o)
                    return pts

                def emit_pv(j, pts):
                    vslice = vt[:, 512 * j + 128 * h:512 * j + 128 * (h + 1)]
                    for s, (pt, o) in pts.items():
                        nc.tensor.matmul(
                            otp[s][:, o:], lhsT=vslice, rhs=pt[:, o:],
                            start=(j == 0), stop=(j == 4 * s + 3))
                    for s, (pt, o) in pts.items():
                        if j == 0:
                            nc.vector.tensor_copy(rts[s], pt)
                        else:
                            nc.vector.tensor_add(
                                rts[s][:, o:], rts[s][:, o:], pt[:, o:])
                    # strip finalize: denominator, reciprocal, normalize
                    for s, (pt, o) in pts.items():
                        if j != 4 * s + 3:
                            continue
                        sump = psum.tile([128, 512], F32, tag="ps", bufs=4,
                                         name=f"sump_{h}_{s}")
                        nc.tensor.matmul(sump, lhsT=ones, rhs=rts[s],
                                         start=True, stop=True)
                        rin = work.tile([128, 512], F32, tag="rin", bufs=2,
                                        name=f"rin_{h}_{s}")
                        nc.vector.reciprocal(rin, sump)
                        t0 = 512 * s
                        nc.vector.tensor_mul(
                            ot[:, T * h + t0:T * h + t0 + 512], otp[s], rin)

                prev = None
                for j in range(4 * NSTRIP):
                    cur = emit_s(j)
                    if prev is not None:
                        emit_pv(j - 1, prev)
                    prev = cur
                emit_pv(4 * NSTRIP - 1, prev)

            # ---- projection  yT[cout, t] = wp-slices.T x ot-strips ----
            for cb in range(16):            # cout 128-blocks
                cs = cb // 4
                cbo = 128 * (cb % 4)
                ypps = [psum.tile([128, 512], F32, tag="otp", bufs=4,
                                  name=f"yp_{cb}_{s}") for s in range(NSTRIP)]
                for hp in range(HLOC):
                    w_cb = wpt[(hp, cs)][:, cbo:cbo + 128]
                    for s in range(NSTRIP):
                        nc.tensor.matmul(
                            ypps[s], lhsT=w_cb,
                            rhs=ot[:, T * hp + 512 * s:T * hp + 512 * (s + 1)],
                            start=(hp == 0), stop=(hp == HLOC - 1))
                for s in range(NSTRIP):
                    ysb = work.tile([128, 512], FP16, tag="ysb", bufs=4,
                                    name=f"ysb_{cb}_{s}")
                    if (cb + s) % 2 == 0:
                        nc.vector.tensor_copy(ysb, ypps[s])
                    else:
                        nc.scalar.copy(ysb, ypps[s])
                    nc.sync.dma_start(
                        out=yt[128 * cb:128 * (cb + 1),
                               512 * s:512 * (s + 1)],
                        in_=ysb)

            if reps > 1:
                loop_ctx.__exit__(None, None, None)

    nc.compile()
    _strip_redundant_ldweights(nc)
    return nc


def _strip_redundant_ldweights(nc):
    """Remove back-to-back InstLdweights that reload the exact weights already
    resident in the PE array (legalization emits one per matmul with no dedup;
    each serialized reload costs ~53-107ns on HW).  Only sync-free loads whose
    (weights AP, perf_mode, tile_position) matches the immediately preceding
    PE weight state are dropped; weight state is conservatively reset at block
    boundaries and on any non-matmul PE instruction."""

    def ap_sig(ap):
        try:
            return ap.to_json()
        except Exception:
            return repr(ap)

    for blk in nc.m.functions[0].blocks:
        cur = None
        keep = []
        changed = False
        for inst in blk.instructions:
            if getattr(inst, "engine", None) != mybir.EngineType.PE:
                keep.append(inst)
                continue
            nm = inst.__class__.__name__
            if nm == "InstLdweights":
                sig = (ap_sig(inst.ins[0]), getattr(inst, "perf_mode", None),
                       getattr(inst, "tile_position", None))
                si = inst.sync_info
                sync_free = not (si and (si.on_wait or si.on_update))
                if sig == cur and sync_free:
                    changed = True
                    continue
                cur = sig
            elif nm != "InstMatmult":
                cur = None
            keep.append(inst)
        if changed:
            blk.instructions = keep


def _host_inputs(x, w_attn, w_proj):
    """Per-core input dicts."""
    x = np.asarray(x, dtype=np.float32)
    w_attn = np.asarray(w_attn, dtype=np.float32)
    w_proj = np.asarray(w_proj, dtype=np.float32)

    p = np.arange(128)[:, None]
    f = np.arange(128)[None, :]
    mask01 = np.where(p <= f, 1.0, 0.0).astype(np.float16)
    ones = np.ones((128, 128), dtype=np.float16)

    in_maps = []
    for core in range(NCORES):
        b, g = divmod(core, 4)
        r0 = 512 * g
        wq = w_attn[r0:r0 + 512, :]            # [512, C]
        wk = w_attn[C + r0:C + r0 + 512, :]
        wv = w_attn[2 * C + r0:2 * C + r0 + 512, :]
        wqkv = np.ascontiguousarray(
            np.concatenate([wq.T, wk.T, wv.T], axis=1)).astype(
                np.float16)                    # [C, 1536]
        wpm = np.ascontiguousarray(w_proj[:, r0:r0 + 512].T).astype(
            np.float16)                        # [512, C]
        in_maps.append({
            "xt": np.ascontiguousarray(x[b].T).astype(np.float16),
            "wqkv": wqkv,
            "wp": wpm,
            "mask01_in": mask01,
            "ones_in": ones,
        })
    return in_maps


def kernel(x, w_attn, w_proj, b_proj):
    if "nc" not in _cache:
        _cache["nc"] = _build_nc()
    nc = _cache["nc"]

    in_maps = _host_inputs(x, w_attn, w_proj)
    res = run_bass_kernel_spmd(nc, in_maps, core_ids=list(range(NCORES)))
    _cache["last_result"] = res
    if res.exec_time_ns is not None:
        print(f"HW exec time: {res.exec_time_ns} ns")

    b_proj = np.asarray(b_proj, dtype=np.float32)
    out = np.empty((B, T, C), dtype=np.float32)
    for b in range(B):
        acc = res.results[4 * b]["yt"].astype(np.float32)
        for g in range(1, 4):
            acc = acc + res.results[4 * b + g]["yt"].astype(np.float32)
        out[b] = acc.T + b_proj[None, :]
    return out


# revision 9
# speedup vs baseline: 1.1183x; 1.1183x over previous
"""Causal multi-head attention layer (B=2, T=2048, C=2048, H=16) on 8 TRN2
NeuronCores — v5: fp16, pair-strip attention with fused 2-bank psum tiles,
DVE softmax-denominator accumulation, weight-stationary v with PE transpose.
(v1 fp32r: 487675ns; v2 bf16: 454541ns; v3: 418466ns; v4: ~446000ns.)

Sharding: data-parallel over batch (2 groups of 4 cores), tensor-parallel over
heads within a group (4 heads/core, Megatron column-split of w_attn and
row-split of w_proj).  Each core computes a partial projection output in
transposed layout yT = (O_heads @ w_proj[:, cols].T).T; the host transposes,
sums the 4 partials per batch element and adds b_proj.

v5 structure:
  - all operands fp16 (better mantissa than bf16, same PE speed, unlocks DVE
    2x 2-byte modes).
  - attention in strip pairs; each chunk's two S matmuls write the two bank
    halves of ONE [128,1024] psum tile, so exp / R-accumulate / PV consume it
    with ONE Act instruction, ONE DVE add and two PV matmuls per chunk.
  - softmax denominator: DVE accumulates exp chunks into R2 (fp16, 2x mode);
    one ones-matmul per (head, strip) + reciprocal + normalize on DVE.
    This removes the per-chunk ones-matmul of v3 (~70K PE cycles + 96 loads).
  - causal diagonal handled by multiplying the exp output block with a 0/1
    fp16 mask on DVE (no -1e30 f32 adds in psum).
  - v computed weight-stationary into vT psum (64 LDWEIGHTS instead of 256),
    then transposed back 128x128-wise on the PE (64 transposes) with Act
    doing all psum->sbuf copies (idle during the qkv phase).
  - q/k run in u-pairs (8 psum accumulators: 4x 2KB + 2x 4KB-halves) so the
    first x sweep covers the x DMA; w_k/w_v chunk DMAs deferred after x+w_q.
  - PSUM = tag "otp" 4 x [128,512] + tag "st2" 2 x [128,1024] = 16KB exactly.
"""

import numpy as np

import concourse.bacc as bacc
import concourse.tile as tile
from concourse import mybir
from concourse.bass_utils import run_bass_kernel_spmd

F32 = mybir.dt.float32
FP16 = mybir.dt.float16

B, T, C, H = 2, 2048, 2048, 16
HD = C // H            # 128
HLOC = 4               # heads per core
NCORES = 8
NSTRIP = T // 512      # 4 t-strips
NCH = C // 128         # 16 contraction chunks
SCALE = 1.0 / float(np.sqrt(HD))

_cache = {}


def _build_nc(reps=1):
    nc = bacc.Bacc("TRN2", debug=False)

    xt = nc.dram_tensor("xt", [C, T], FP16, kind="ExternalInput")      # x[b].T
    wqkv = nc.dram_tensor("wqkv", [C, 3 * 512], FP16, kind="ExternalInput")
    wp = nc.dram_tensor("wp", [512, C], FP16, kind="ExternalInput")
    mask01_in = nc.dram_tensor("mask01_in", [128, 128], FP16,
                               kind="ExternalInput")
    ones_in = nc.dram_tensor("ones_in", [128, 128], FP16, kind="ExternalInput")
    id_in = nc.dram_tensor("id_in", [128, 128], FP16, kind="ExternalInput")
    yt = nc.dram_tensor("yt", [C, T], FP16, kind="ExternalOutput")

    with tile.TileContext(nc) as tc:
        with (
            tc.tile_pool(name="persist", bufs=1) as persist,
            tc.tile_pool(name="work", bufs=2) as work,
            tc.tile_pool(name="psum", bufs=4, space="PSUM") as psum,
        ):
            qt = persist.tile([128, HLOC * T], FP16, tag="qt")
            kt = persist.tile([128, HLOC * T], FP16, tag="kt")
            vt = persist.tile([128, HLOC * T], FP16, tag="vt")
            ot = persist.tile([128, HLOC * T], FP16, tag="ot")
            mask01 = persist.tile([128, 128], FP16, tag="mask01")
            ones = persist.tile([128, 128], FP16, tag="ones")
            idm = persist.tile([128, 128], FP16, tag="idm")
            nc.sync.dma_start(out=mask01, in_=mask01_in[:, :])
            nc.sync.dma_start(out=ones, in_=ones_in[:, :])
            nc.sync.dma_start(out=idm, in_=id_in[:, :])

            if reps > 1:
                loop_ctx = tc.For_i(
                    0, reps, 1,
                    hint_engines=(mybir.EngineType.PE,
                                  mybir.EngineType.DVE,
                                  mybir.EngineType.Activation,
                                  mybir.EngineType.SP,
                                  mybir.EngineType.Pool))
                loop_ctx.__enter__()

            # ---- DMA: x + w_q interleaved first (q pass is DMA-gated),
            # then w_k, then w_v ----
            xcs = {}
            wts = {}
            for cc in range(NCH):
                wt = work.tile([128, 512], FP16, tag="wch", bufs=32,
                               name=f"w_0_{cc}")
                nc.sync.dma_start(
                    out=wt, in_=wqkv[128 * cc:128 * (cc + 1), 0:512])
                wts[(0, cc)] = wt
                xcw = work.tile([128, T], FP16, tag="xc", bufs=16,
                                name=f"xc_{cc}")
                nc.sync.dma_start(
                    out=xcw, in_=xt[128 * cc:128 * (cc + 1), :])
                for s in range(NSTRIP):
                    xcs[(cc, s)] = xcw[:, 512 * s:512 * s + 512]
            for pas in (1, 2):
                for cc in range(NCH):
                    wt = work.tile([128, 512], FP16, tag="wch", bufs=32,
                                   name=f"w_{pas}_{cc}")
                    nc.sync.dma_start(
                        out=wt, in_=wqkv[128 * cc:128 * (cc + 1),
                                         512 * pas:512 * (pas + 1)])
                    wts[(pas, cc)] = wt

            # ---- q/k: weight-stationary, u-pairs, full-K psum accumulation
            for pas in range(2):            # 0=q, 1=k
                dst = qt if pas == 0 else kt
                for up in range(2):         # u-pair = 2 head-blocks at a time
                    acc2 = [psum.tile([128, 1024], F32, tag="st2", bufs=2,
                                      name=f"acc2_{pas}_{up}_{i}")
                            for i in range(2)]
                    accs = {}
                    for s in range(NSTRIP):
                        accs[(0, s)] = psum.tile(
                            [128, 512], F32, tag="otp", bufs=4,
                            name=f"acc_{pas}_{up}_{s}")
                        accs[(1, s)] = acc2[s // 2][:, 512 * (s % 2):
                                                    512 * (s % 2) + 512]
                    for cc in range(NCH):
                        for u2 in range(2):
                            u = 2 * up + u2
                            w_u = wts[(pas, cc)][:, 128 * u:128 * (u + 1)]
                            for s in range(NSTRIP):
                                nc.tensor.matmul(
                                    accs[(u2, s)], lhsT=w_u, rhs=xcs[(cc, s)],
                                    start=(cc == 0), stop=(cc == NCH - 1))
                    for u2 in range(2):
                        u = 2 * up + u2
                        for s in range(NSTRIP):
                            dslice = dst[:, T * u + 512 * s:
                                         T * u + 512 * (s + 1)]
                            if (u2 + s) % 2 == 0:
                                nc.scalar.copy(dslice, accs[(u2, s)])
                            else:
                                nc.vector.tensor_copy(dslice, accs[(u2, s)])

            # ---- v: weight-stationary vT, then PE-transpose to vt ----
            for u in range(4):              # vchan 128-block
                vaccs = [psum.tile([128, 512], F32, tag="otp", bufs=4,
                                   name=f"vacc_{u}_{s}")
                         for s in range(NSTRIP)]
                for cc in range(NCH):
                    w_u = wts[(2, cc)][:, 128 * u:128 * (u + 1)]
                    for s in range(NSTRIP):
                        nc.tensor.matmul(
                            vaccs[s], lhsT=w_u, rhs=xcs[(cc, s)],
                            start=(cc == 0), stop=(cc == NCH - 1))
                for s in range(NSTRIP):
                    vsb = work.tile([128, 512], FP16, tag="vtsb", bufs=2,
                                    name=f"vsb_{u}_{s}")
                    nc.scalar.copy(vsb, vaccs[s])
                    for tb in range(4):
                        j = 4 * s + tb
                        tp = psum.tile([128, 128], FP16, tag="otp", bufs=4,
                                       name=f"vtp_{u}_{j}")
                        nc.tensor.transpose(
                            tp, vsb[:, 128 * tb:128 * (tb + 1)], idm)
                        nc.scalar.copy(
                            vt[:, 512 * j + 128 * u:512 * j + 128 * (u + 1)],
                            tp)

            # ---- proj weights: DMA early so they arrive during attention ----
            wpt = {}
            for hp in range(HLOC):
                wtw = work.tile([128, T], FP16, tag="xc", bufs=16,
                                name=f"wpt_{hp}")
                nc.sync.dma_start(
                    out=wtw, in_=wp[128 * hp:128 * (hp + 1), :])
                for cs in range(4):
                    wpt[(hp, cs)] = wtw[:, 512 * cs:512 * (cs + 1)]

            # ---- attention: strip pairs, fused 2-bank psum per chunk ----
            for sp in range(NSTRIP // 2):
                sa, sb = 2 * sp, 2 * sp + 1
                nj = 4 * (sb + 1)
                for h in range(HLOC):
                    otp = {s: psum.tile([128, 512], F32, tag="otp", bufs=4,
                                        name=f"otp_{sp}_{h}_{s}")
                           for s in (sa, sb)}
                    r2 = work.tile([128, 1024], FP16, tag="rt", bufs=4,
                                   name=f"r2_{sp}_{h}")

                    def emit_s(j):
                        """S matmuls (both strips into one 2-bank psum tile),
                        one exp, diag mask."""
                        smin = j // 4
                        kslice = kt[:, T * h + 128 * j:T * h + 128 * (j + 1)]
                        st2 = psum.tile([128, 1024], F32, tag="st2", bufs=2,
                                        name=f"st2_{sp}_{h}_{j}")
                        pt2 = work.tile([128, 1024], FP16, tag="pt", bufs=4,
                                        name=f"pt2_{sp}_{h}_{j}")
                        segs = {}
                        for s in (sa, sb):
                            if j >= 4 * (s + 1):
                                continue
                            base = 0 if s == sa else 512
                            o = 128 * (j - 4 * s) if s == smin else 0
                            t0 = 512 * s
                            nc.tensor.matmul(
                                st2[:, base + o:base + 512], lhsT=kslice,
                                rhs=qt[:, T * h + t0 + o:T * h + t0 + 512],
                                start=True, stop=True)
                            segs[s] = (base, o)
                        lo = min(base + o for base, o in segs.values())
                        nc.scalar.activation(
                            pt2[:, lo:], st2[:, lo:],
                            mybir.ActivationFunctionType.Exp,
                            scale=SCALE)
                        if smin in segs:
                            base, o = segs[smin]
                            pos = base + o
                            nc.vector.tensor_mul(
                                pt2[:, pos:pos + 128], pt2[:, pos:pos + 128],
                                mask01)
                        return pt2, segs, lo

                    def emit_pv(j, info):
                        pt2, segs, lo = info
                        vslice = vt[:, 512 * j + 128 * h:
                                    512 * j + 128 * (h + 1)]
                        for s, (base, o) in segs.items():
                            nc.tensor.matmul(
                                otp[s][:, o:], lhsT=vslice,
                                rhs=pt2[:, base + o:base + 512],
                                start=(j == 0), stop=(j == 4 * s + 3))
                        if j == 0:
                            nc.vector.tensor_copy(r2, pt2)
                        else:
                            nc.vector.tensor_add(
                                r2[:, lo:], r2[:, lo:], pt2[:, lo:])
                        for s, (base, o) in segs.items():
                            if j != 4 * s + 3:
                                continue
                            # strip finalize: denominator + normalize
                            sump = psum.tile([128, 512], F32, tag="otp",
                                             bufs=4, name=f"sump_{sp}_{h}_{s}")
                            nc.tensor.matmul(
                                sump, lhsT=ones, rhs=r2[:, base:base + 512],
                                start=True, stop=True)
                            rin = work.tile([128, 512], F32, tag="rin",
                                            bufs=2, name=f"rin_{sp}_{h}_{s}")
                            nc.vector.reciprocal(rin, sump)
                            t0 = 512 * s
                            nc.vector.tensor_mul(
                                ot[:, T * h + t0:T * h + t0 + 512],
                                otp[s], rin)

                    prev = None
                    for j in range(nj):
                        cur = emit_s(j)
                        if prev is not None:
                            emit_pv(j - 1, prev)
                        prev = cur
                    emit_pv(nj - 1, prev)

            # ---- projection  yT[cout, t] = wp-slices.T x ot-strips ----
            for cb in range(16):            # cout 128-blocks
                cs = cb // 4
                cbo = 128 * (cb % 4)
                ypps = [psum.tile([128, 512], F32, tag="otp", bufs=4,
                                  name=f"yp_{cb}_{s}") for s in range(NSTRIP)]
                for hp in range(HLOC):
                    w_cb = wpt[(hp, cs)][:, cbo:cbo + 128]
                    for s in range(NSTRIP):
                        nc.tensor.matmul(
                            ypps[s], lhsT=w_cb,
                            rhs=ot[:, T * hp + 512 * s:T * hp + 512 * (s + 1)],
                            start=(hp == 0), stop=(hp == HLOC - 1))
                ysb = work.tile([128, T], FP16, tag="ysb", bufs=2,
                                name=f"ysb_{cb}")
                for s in range(NSTRIP):
                    if (cb + s) % 2 == 0:
                        nc.vector.tensor_copy(
                            ysb[:, 512 * s:512 * (s + 1)], ypps[s])
                    else:
                        nc.scalar.copy(
                            ysb[:, 512 * s:512 * (s + 1)], ypps[s])
                nc.sync.dma_start(
                    out=yt[128 * cb:128 * (cb + 1), :], in_=ysb)

            if reps > 1:
                loop_ctx.__exit__(None, None, None)

    nc.compile()
    _strip_redundant_ldweights(nc)
    return nc


def _strip_redundant_ldweights(nc):
    """Remove back-to-back InstLdweights that reload the exact weights already
    resident in the PE array (legalization emits one per matmul with no dedup;
    each serialized reload costs ~53-107ns on HW).  Only sync-free loads whose
    (weights AP, perf_mode, tile_position) matches the immediately preceding
    PE weight state are dropped; weight state is conservatively reset at block
    boundaries and on any non-matmul PE instruction."""

    def ap_sig(ap):
        try:
            return ap.to_json()
        except Exception:
            return repr(ap)

    for blk in nc.m.functions[0].blocks:
        cur = None
        keep = []
        changed = False
        for inst in blk.instructions:
            if getattr(inst, "engine", None) != mybir.EngineType.PE:
                keep.append(inst)
                continue
            nm = inst.__class__.__name__
            if nm == "InstLdweights":
                sig = (ap_sig(inst.ins[0]), getattr(inst, "perf_mode", None),
                       getattr(inst, "tile_position", None))
                si = inst.sync_info
                sync_free = not (si and (si.on_wait or si.on_update))
                if sig == cur and sync_free:
                    changed = True
                    continue
                cur = sig
            elif nm != "InstMatmult":
                cur = None
            keep.append(inst)
        if changed:
            blk.instructions = keep


def _host_inputs(x, w_attn, w_proj):
    """Per-core input dicts."""
    x = np.asarray(x, dtype=np.float32)
    w_attn = np.asarray(w_attn, dtype=np.float32)
    w_proj = np.asarray(w_proj, dtype=np.float32)

    p = np.arange(128)[:, None]
    f = np.arange(128)[None, :]
    mask01 = np.where(p <= f, 1.0, 0.0).astype(np.float16)
    ones = np.ones((128, 128), dtype=np.float16)
    idm = np.eye(128, dtype=np.float16)

    in_maps = []
    for core in range(NCORES):
        b, g = divmod(core, 4)
        r0 = 512 * g
        wq = w_attn[r0:r0 + 512, :]            # [512, C]
        wk = w_attn[C + r0:C + r0 + 512, :]
        wv = w_attn[2 * C + r0:2 * C + r0 + 512, :]
        wqkv = np.ascontiguousarray(
            np.concatenate([wq.T, wk.T, wv.T], axis=1)).astype(
                np.float16)                    # [C, 1536]
        wpm = np.ascontiguousarray(w_proj[:, r0:r0 + 512].T).astype(
            np.float16)                        # [512, C]
        in_maps.append({
            "xt": np.ascontiguousarray(x[b].T).astype(np.float16),
            "wqkv": wqkv,
            "wp": wpm,
            "mask01_in": mask01,
            "ones_in": ones,
            "id_in": idm,
        })
    return in_maps


def kernel(x, w_attn, w_proj, b_proj):
    if "nc" not in _cache:
        _cache["nc"] = _build_nc()
    nc = _cache["nc"]

    in_maps = _host_inputs(x, w_attn, w_proj)
    res = run_bass_kernel_spmd(nc, in_maps, core_ids=list(range(NCORES)))
    _cache["last_result"] = res
    if res.exec_time_ns is not None:
        print(f"HW exec time: {res.exec_time_ns} ns")

    b_proj = np.asarray(b_proj, dtype=np.float32)
    out = np.empty((B, T, C), dtype=np.float32)
    for b in range(B):
        acc = res.results[4 * b]["yt"].astype(np.float32)
        for g in range(1, 4):
            acc = acc + res.results[4 * b + g]["yt"].astype(np.float32)
        out[b] = acc.T + b_proj[None, :]
    return out
